# revision 1
# baseline (speedup 1.0000x reference)
"""Trainium2 Bass kernel for nn_ASISNativeAttention (B=2,S=2048,D=1024,H=16).

Sharding: 8 cores = 2 batches x 4 head-groups (4 heads each); host splits
inputs per core and sums the 4 partial output projections per batch (+bo).

Per-core pipeline (single fused TileContext phase, all matmuls bf16 with
fp32 PSUM; softmax kept fp32 through the exp):
  xT     via PE transposes (identity passed as input), drained by ONE wide
         strided DVE copy per tile; mean-pool via a ones-column matmul
         accumulated into a [1,1024] PSUM row, feeding the sigmoid gates
  qT,kT  [256,2048] transposed projections (bias + 1/8 fold into q copy)
  v      [2048, 4x65] natural projection; rank-1 matmul adds bv; a ones
         column per head makes PV emit softmax denominators for free
  scoresT[sk,sq] per head; NO max pass (scores bounded ~5); TWO sk-tiles
         of scores share one [128,1024] PSUM tile so ONE wide ACT exp
         covers both (ACT is the bottleneck engine)
  ctx    = PV / denominator (per-partition scalar); gate (ethics*safety,
         broadcast via tiny matmuls) folds into the ctx transpose copy
  out    = ctxT^T-slices @ Wo_slice, partial, summed on host

Emission-order software pipelining keeps ACT (exp) saturated: x-load /
kq0 / chunk-0 scores interleaved in a lead-in; each chunk's tail
(PV-b, normalize, ctxT) is emitted after the NEXT chunk's head-a scores;
deferred qk1/out-proj work is spread as rate-matched filler units.
A post-pass hoists multi-wait sync conditions onto single-wait NoOps
(this walrus build rejects >1 sync wait on most instruction encodings).

kernel.py is self-contained: only needs numpy/ml_dtypes + the concourse
tree at /opt/trn_rl_repo. Modeled per-core device time: ~209 us.
"""

import os
import sys
import numpy as np
import ml_dtypes

BF16 = ml_dtypes.bfloat16

sys.path.insert(0, "/opt/trn_rl_repo")

B, S, D, H = 2, 2048, 1024, 16
HD = 64          # head dim
NCORES = 8
HG = 4           # head groups = cores per batch
HL = H // HG     # heads per core (4)
DL = D // HG     # local width (256)
ST = S // 128    # 16 s-tiles
DT = D // 128    # 8 d-tiles
SC = 512         # sq chunk width for scores
NSC = S // SC    # 4 chunks

_CACHE = {}


def _build_nc():
    import concourse.bass as bass
    import concourse.mybir as mybir
    from concourse.tile import TileContext

    fp32 = mybir.dt.float32
    bf16 = mybir.dt.bfloat16
    AF = mybir.ActivationFunctionType
    ALU = mybir.AluOpType

    nc = bass.Bass()

    x_d = nc.declare_dram_parameter("x", [S, D], bf16, isOutput=False)
    wq_d = nc.declare_dram_parameter("wq", [128, DT * DL], bf16, isOutput=False)
    wk_d = nc.declare_dram_parameter("wk", [128, DT * DL], bf16, isOutput=False)
    wv_d = nc.declare_dram_parameter("wv", [128, DT * DL], bf16, isOutput=False)
    wo_d = nc.declare_dram_parameter("wo", [128, 2 * D], bf16, isOutput=False)
    bq_d = nc.declare_dram_parameter("bq2", [128, 2], fp32, isOutput=False)
    bk_d = nc.declare_dram_parameter("bk2", [128, 2], fp32, isOutput=False)
    bv_d = nc.declare_dram_parameter("bvrow", [1, DL], bf16, isOutput=False)
    wes_d = nc.declare_dram_parameter("wes", [128, DT * 2 * HL], fp32, isOutput=False)
    bes_d = nc.declare_dram_parameter("bes", [HL, 2], fp32, isOutput=False)
    gexp_d = nc.declare_dram_parameter("gexp", [HL, DL], fp32, isOutput=False)
    id_d = nc.declare_dram_parameter("ident", [128, 128], bf16, isOutput=False)
    out_d = nc.declare_dram_parameter("out", [S, D], fp32, isOutput=True)

    with TileContext(nc) as tc:
        with tc.tile_pool(name="persist", bufs=1) as P:
            # ---- persistent SBUF tensors ----
            xT = P.tile([128, DT * S], bf16, tag="xT")
            wq = P.tile([128, DT * DL], bf16, tag="wq")
            wk = P.tile([128, DT * DL], bf16, tag="wk")
            wv = P.tile([128, DT * DL], bf16, tag="wv")
            wo = P.tile([128, 2 * D], bf16, tag="wo")
            qT = P.tile([128, 2 * S], bf16, tag="qT")
            kT = P.tile([128, 2 * S], bf16, tag="kT")
            v = P.tile([128, ST * HL * 65], bf16, tag="v")
            ctx = P.tile([128, ST * DL], bf16, tag="ctx")
            ctxT = P.tile([128, 2 * S], bf16, tag="ctxT")
            bq2 = P.tile([128, 2], fp32, tag="bq2")
            bk2 = P.tile([128, 2], fp32, tag="bk2")
            bvrow = P.tile([1, DL], bf16, tag="bvrow")
            wes = P.tile([128, DT * 2 * HL], fp32, tag="wes")
            bes = P.tile([HL, 2], fp32, tag="bes")
            gexp = P.tile([HL, DL], fp32, tag="gexp")
            ident = P.tile([128, 128], bf16, tag="ident")
            ones_row = P.tile([1, 128], bf16, tag="ones_row")
            ones_col = P.tile([128, 1], bf16, tag="ones_col")
            ones1 = P.tile([1, 1], fp32, tag="ones1")
            xmrow = P.tile([1, D], fp32, tag="xmrow")
            xm_col = P.tile([128, DT], fp32, tag="xm_col")
            gcol = P.tile([128, 2], fp32, tag="gcol")

            dma = nc.sync.dma_start
            _dma_engines = [nc.sync, nc.scalar, nc.gpsimd]
            _dma_i = [0]

            def dma_rr(out, in_):
                eng = _dma_engines[_dma_i[0] % len(_dma_engines)]
                _dma_i[0] += 1
                eng.dma_start(out=out, in_=in_)
            nc.vector.memset(ones_row[:], 1.0)
            nc.vector.memset(ones_col[:], 1.0)
            nc.vector.memset(ones1[:], 1.0)

            # ---- phase A/C fused: pipelined load + QKV + attention + out-proj ----
            with (
                tc.tile_pool(name="xload", bufs=6) as XL,
                tc.tile_pool(name="pmm", bufs=2, space="PSUM") as PM,
                tc.tile_pool(name="pc", bufs=4, space="PSUM") as PC,
                tc.tile_pool(name="ex", bufs=28) as EX,
                tc.tile_pool(name="rc", bufs=8) as RC,
                tc.tile_pool(name="ob", bufs=4) as OB,
                tc.tile_pool(name="gs", bufs=1) as GS,
            ):
                xmp_ps = [PC.tile([1, D // 2], fp32, tag="cp", name=f"xmp_ps{h}")
                          for h in range(2)]

                def load_x_tile(t):
                    xb = XL.tile([128, D], bf16, tag="xb", name=f"xb{t}")
                    dma_rr(xb[:], x_d[t * 128:(t + 1) * 128, :])
                    for hf in range(2):
                        nc.tensor.matmul(
                            xmp_ps[hf][:], lhsT=ones_col[:],
                            rhs=xb[:, hf * (D // 2):(hf + 1) * (D // 2)],
                            start=(t == 0), stop=(t == ST - 1),
                        )
                    ps = PM.tile([128, 1024], bf16, tag="mm", name=f"tr{t}")
                    for j in range(DT):
                        nc.tensor.transpose(
                            ps[:, j * 128:(j + 1) * 128],
                            xb[:, j * 128:(j + 1) * 128], ident[:])
                    nc.vector.tensor_copy(
                        xT.rearrange("p (j s) -> p j s", s=S)[:, :, t * 128:(t + 1) * 128],
                        ps.rearrange("p (j c) -> p j c", c=128)[:, :, :],
                    )

                def qk_units(w, dst, bias, scl, i, sc):
                    """One whole qk projection as a single filler unit."""
                    def unit():
                        pp = PC.tile([128, SC], fp32, tag="cp",
                                     name=f"pp{w.tensor.name}_{i}_{sc}")
                        for j in range(DT):
                            nc.tensor.matmul(
                                pp[:],
                                lhsT=w[:, j * DL + i * 128: j * DL + (i + 1) * 128],
                                rhs=xT[:, j * S + sc * SC: j * S + (sc + 1) * SC],
                                start=(j == 0),
                                stop=(j == DT - 1),
                            )
                        nc.vector.tensor_scalar(
                            out=dst[:, i * S + sc * SC: i * S + (sc + 1) * SC],
                            in0=pp[:],
                            scalar1=bias[:, i: i + 1],
                            scalar2=scl,
                            op0=ALU.add,
                            op1=ALU.mult,
                        )
                    return [(1700, unit)]

                def project_v(t):
                    pv = PC.tile([128, DL], fp32, tag="cp", name=f"pv{t}")
                    nc.tensor.matmul(
                        pv[:], lhsT=ones_row[:], rhs=bvrow[:], start=True, stop=False,
                    )
                    for j in range(DT):
                        nc.tensor.matmul(
                            pv[:],
                            lhsT=xT[:, j * S + t * 128: j * S + (t + 1) * 128],
                            rhs=wv[:, j * DL:(j + 1) * DL],
                            start=False,
                            stop=(j == DT - 1),
                        )
                    vt = v[:, t * HL * 65:(t + 1) * HL * 65]
                    nc.vector.memset(
                        vt.rearrange("p (h c) -> p h c", c=65)[:, :, 64:65], 1.0
                    )
                    nc.vector.tensor_copy(
                        vt.rearrange("p (h c) -> p h c", c=65)[:, :, 0:64],
                        pv.rearrange("p (h c) -> p h c", c=64)[:, :, :],
                    )

                def outproj_units(t):
                    state = {}
                    def mk(n):
                        def unit():
                            if n == 0:
                                state["ot"] = OB.tile([128, D], fp32, tag="ot",
                                                      name=f"ot{t}")
                            po = PC.tile([128, 512], fp32, tag="cp", name=f"po{t}_{n}")
                            for i2 in range(2):
                                nc.tensor.matmul(
                                    po[:],
                                    lhsT=ctxT[:, i2 * S + t * 128: i2 * S + (t + 1) * 128],
                                    rhs=wo[:, i2 * D + n * 512: i2 * D + (n + 1) * 512],
                                    start=(i2 == 0), stop=(i2 == 1),
                                )
                            nc.vector.tensor_copy(
                                state["ot"][:, n * 512:(n + 1) * 512], po[:])
                            if n == 1:
                                dma_rr(out_d[t * 128:(t + 1) * 128, :], state["ot"][:])
                        return (426, unit)
                    return [mk(0), mk(1)]

                fillers = []

                def pop_fillers(budget_ns):
                    spent = 0
                    while fillers and spent < budget_ns:
                        ns, unit = fillers.pop(0)
                        unit()
                        spent += ns

                def score_exp(i, sc, hh, skp):
                    """Scores for sk-tiles (2*skp, 2*skp+1), one wide exp."""
                    h = 2 * i + hh
                    r = hh * 64
                    sp = PM.tile([128, 2 * SC], fp32, tag="mm",
                                 name=f"sp{i}_{sc}_{h}_{skp}")
                    for half in range(2):
                        sk = 2 * skp + half
                        nc.tensor.matmul(
                            sp[:, half * SC:(half + 1) * SC],
                            lhsT=kT[r:r + 64, i * S + sk * 128: i * S + (sk + 1) * 128],
                            rhs=qT[r:r + 64, i * S + sc * SC: i * S + (sc + 1) * SC],
                            start=True, stop=True,
                        )
                    et = EX.tile([128, 2 * SC], bf16, tag="et",
                                 name=f"et{i}_{sc}_{h}_{skp}")
                    nc.scalar.activation(et[:], sp[:], AF.Exp)
                    return et

                def pv_mm(cps, ets, i, hh, sk):
                    h = 2 * i + hh
                    skp, half = sk // 2, sk % 2
                    for u in range(SC // 128):
                        nc.tensor.matmul(
                            cps[u][:, hh * 65:(hh + 1) * 65],
                            lhsT=ets[skp][:, half * SC + u * 128: half * SC + (u + 1) * 128],
                            rhs=v[:, sk * HL * 65 + h * 65: sk * HL * 65 + (h + 1) * 65],
                            start=(sk == 0),
                            stop=(sk == ST - 1),
                        )

                # ---- lead-in: x pipeline + kq0 + scores of chunk (i=0, sc=0) ----
                dma_rr(ident[:], id_d[:])
                ets_a0 = []
                ets_b0 = []
                for g in range(4):
                    for t in range(4 * g, 4 * g + 4):
                        load_x_tile(t)
                    if g == 0:
                        dma_rr(wk[:], wk_d[:])
                        dma_rr(wq[:], wq_d[:])
                        dma_rr(wv[:], wv_d[:])
                        dma(bq2[:], bq_d[:])
                        dma(bk2[:], bk_d[:])
                        dma(bvrow[:], bv_d[:])
                        dma(wes[:], wes_d[:])
                        dma(bes[:], bes_d[:])
                        dma(gexp[:], gexp_d[:])
                    if g == 1:
                        dma_rr(wo[:], wo_d[:])
                    for us in (qk_units(wk, kT, bk2, 1.0, 0, g),
                               qk_units(wq, qT, bq2, 0.125, 0, g)):
                        for _, unit in us:
                            unit()
                    # chunk-0 head-a scores for this group's kT columns
                    for skp in range(2 * g, 2 * g + 2):
                        ets_a0.append(score_exp(0, 0, 0, skp))
                # gates (tiny; needs the full x mean row)
                for hf in range(2):
                    nc.vector.tensor_copy(
                        xmrow[:, hf * (D // 2):(hf + 1) * (D // 2)], xmp_ps[hf][:])
                xcp = PC.tile([128, DT], fp32, tag="cp", name="xcp")
                for j in range(DT):
                    nc.tensor.matmul(
                        xcp[:, j: j + 1],
                        lhsT=xmrow[:, j * 128:(j + 1) * 128],
                        rhs=ones1[:],
                        start=True, stop=True,
                    )
                nc.vector.tensor_copy(xm_col[:], xcp[:])
                gpe = PC.tile([HL, 1], fp32, tag="cp", name="gpe")
                gps = PC.tile([HL, 1], fp32, tag="cp", name="gps")
                for j in range(DT):
                    nc.tensor.matmul(
                        gpe[:], lhsT=wes[:, j * 8: j * 8 + 4],
                        rhs=xm_col[:, j: j + 1],
                        start=(j == 0), stop=(j == DT - 1),
                    )
                for j in range(DT):
                    nc.tensor.matmul(
                        gps[:], lhsT=wes[:, j * 8 + 4: j * 8 + 8],
                        rhs=xm_col[:, j: j + 1],
                        start=(j == 0), stop=(j == DT - 1),
                    )
                eth = GS.tile([HL, 1], fp32, tag="eth")
                saf = GS.tile([HL, 1], fp32, tag="saf")
                gate = GS.tile([HL, 1], fp32, tag="gate")
                nc.scalar.activation(eth[:], gpe[:], AF.Sigmoid, bias=bes[:, 0:1], scale=1.0 / S)
                nc.scalar.activation(saf[:], gps[:], AF.Sigmoid, bias=bes[:, 1:2], scale=1.0 / S)
                nc.vector.tensor_mul(gate[:], eth[:], saf[:])
                for i in range(2):
                    pgc = PC.tile([128, 1], fp32, tag="cp", name=f"pgc{i}")
                    nc.tensor.matmul(
                        pgc[:], lhsT=gexp[:, i * 128:(i + 1) * 128], rhs=gate[:],
                        start=True, stop=True,
                    )
                    nc.vector.tensor_copy(gcol[:, i: i + 1], pgc[:])

                # queue deferred work: v tiles (needed from first PV batch)
                # go inline in chunk 0's b-stretch; kq1 spread over i=0 chunks.
                for s2 in range(NSC):
                    fillers.extend(qk_units(wk, kT, bk2, 1.0, 1, s2))
                for s2 in range(NSC):
                    fillers.extend(qk_units(wq, qT, bq2, 0.125, 1, s2))

                # ---- main chunk loop (chunk-level software pipeline:
                # tail(n-1) is emitted after a-stretch(n) so its serial
                # normalize/out-proj chain hides behind fresh exp work) ----
                pending_tail = [None]

                def make_tail(i, sc, cps_in, ets_b, last, ets_a=None):
                    def tail():
                        cps = cps_in
                        if cps is None:
                            cps = [PC.tile([128, 2 * 65], fp32, tag="cp",
                                           name=f"cp{i}_{sc}_{u}")
                                   for u in range(SC // 128)]
                        for u in range(SC // 128):
                            if ets_a is not None:
                                for sk in range(ST):
                                    skp, half = sk // 2, sk % 2
                                    nc.tensor.matmul(
                                        cps[u][:, 0:65],
                                        lhsT=ets_a[skp][:, half * SC + u * 128: half * SC + (u + 1) * 128],
                                        rhs=v[:, sk * HL * 65 + 2 * i * 65: sk * HL * 65 + (2 * i + 1) * 65],
                                        start=(sk == 0),
                                        stop=(sk == ST - 1),
                                    )
                            h = 2 * i + 1
                            for sk in range(ST):
                                skp, half = sk // 2, sk % 2
                                nc.tensor.matmul(
                                    cps[u][:, 65:130],
                                    lhsT=ets_b[skp][:, half * SC + u * 128: half * SC + (u + 1) * 128],
                                    rhs=v[:, sk * HL * 65 + h * 65: sk * HL * 65 + (h + 1) * 65],
                                    start=(sk == 0),
                                    stop=(sk == ST - 1),
                                )
                            t = sc * (SC // 128) + u
                            for hh in range(2):
                                h2 = 2 * i + hh
                                rec = RC.tile([128, 1], fp32, tag="rec",
                                              name=f"rec{i}_{t}_{h2}")
                                nc.vector.reciprocal(
                                    rec[:], cps[u][:, hh * 65 + 64: hh * 65 + 65])
                                nc.vector.tensor_scalar(
                                    out=ctx[:, t * DL + h2 * HD: t * DL + (h2 + 1) * HD],
                                    in0=cps[u][:, hh * 65: hh * 65 + HD],
                                    scalar1=rec[:],
                                    scalar2=None,
                                    op0=ALU.mult,
                                )
                            tp = PC.tile([128, 128], bf16, tag="cp",
                                         name=f"tp{i}_{t}")
                            nc.tensor.transpose(
                                tp[:],
                                ctx[:, t * DL + i * 128: t * DL + (i + 1) * 128],
                                ident[:],
                            )
                            nc.vector.tensor_scalar(
                                out=ctxT[:, i * S + t * 128: i * S + (t + 1) * 128],
                                in0=tp[:],
                                scalar1=gcol[:, i: i + 1],
                                scalar2=None,
                                op0=ALU.mult,
                            )
                            if i == 1:
                                if last:
                                    for _, unit in outproj_units(t):
                                        unit()
                                else:
                                    fillers.extend(outproj_units(t))
                    return tail

                nchunks = [(i, sc) for i in range(2) for sc in range(NSC)]
                for n, (i, sc) in enumerate(nchunks):
                    first = (n == 0)
                    # a-stretch: scores head a (+ fillers)
                    if first:
                        ets_a = ets_a0
                    else:
                        ets_a = []
                        for skp in range(ST // 2):
                            ets_a.append(score_exp(i, sc, 0, skp))
                            if skp == 7:
                                pop_fillers(3400)
                    # previous chunk's tail, hidden behind this a-stretch
                    if pending_tail[0] is not None:
                        pending_tail[0]()
                        pending_tail[0] = None
                    cps = None
                    if not first:
                        cps = [PC.tile([128, 2 * 65], fp32, tag="cp",
                                       name=f"cp{i}_{sc}_{u}")
                               for u in range(SC // 128)]
                    # b-stretch: scores head b (+ v inline on chunk 0,
                    # whose PV_a is deferred into its tail)
                    ets_b = []
                    for skp in range(ST // 2):
                        ets_b.append(score_exp(i, sc, 1, skp))
                        if first:
                            project_v(2 * skp)
                            project_v(2 * skp + 1)
                        else:
                            pv_mm(cps, ets_a, i, 0, 2 * skp)
                            pv_mm(cps, ets_a, i, 0, 2 * skp + 1)
                    pending_tail[0] = make_tail(i, sc, cps, ets_b,
                                                last=(n == len(nchunks) - 1),
                                                ets_a=(ets_a if first else None))
                pending_tail[0]()

                pop_fillers(10**9)

    _split_multi_waits(nc)
    return nc


def _split_multi_waits(nc, skip=("InstEventSemaphore",)):
    """Hoist extra sync waits onto preceding same-engine NoOps.

    Walrus codegen can attach only one sync wait to some instruction
    encodings (e.g. the PE LDWEIGHTS struct), so any instruction carrying
    N>1 waits is rewritten as N-1 single-wait NoOps followed by the
    instruction with the last wait.
    """
    import concourse.mybir as mybir

    eng = {
        "EngineType.PE": nc.tensor,
        "EngineType.DVE": nc.vector,
        "EngineType.Activation": nc.scalar,
        "EngineType.Pool": nc.gpsimd,
        "EngineType.SP": nc.sync,
    }

    def fresh_nop(engine_key):
        nop = eng[engine_key].nop(hint="wsplit").ins
        for fn in nc.m.functions:
            for bb in fn.blocks:
                for i, ins in enumerate(bb.instructions):
                    if ins.name == nop.name:
                        del bb.instructions[i]
                        return nop
        raise RuntimeError("fresh nop not found")

    for fn in nc.m.functions:
        for bb in fn.blocks:
            insertions = []
            for idx, ins in enumerate(bb.instructions):
                if type(ins).__name__ in skip:
                    continue
                si = ins.sync_info
                if si is None or len(si.on_wait) <= 1:
                    continue
                waits = list(si.on_wait)
                nops = []
                for w in waits[:-1]:
                    nop = fresh_nop(str(ins.engine))
                    nop.sync_info = mybir.SyncInfo(on_wait=[w], on_update=[])
                    nops.append(nop)
                ins.sync_info = mybir.SyncInfo(
                    on_wait=[waits[-1]], on_update=list(si.on_update)
                )
                insertions.append((idx, nops))
            for idx, nops in reversed(insertions):
                bb.instructions[idx:idx] = nops


def _in_maps(inputs):
    x = np.ascontiguousarray(inputs["x"], np.float32)
    maps = []
    ident = np.eye(128, dtype=np.float32)
    gexp = np.zeros((HL, DL), np.float32)
    for h in range(HL):
        gexp[h, h * HD:(h + 1) * HD] = 1.0
    for c in range(NCORES):
        b, g = c // HG, c % HG
        sl = slice(g * DL, (g + 1) * DL)
        hsl = slice(g * HL, (g + 1) * HL)
        wq = inputs["Wq"][:, sl].reshape(DT, 128, DL).transpose(1, 0, 2).reshape(128, DT * DL)
        wk = inputs["Wk"][:, sl].reshape(DT, 128, DL).transpose(1, 0, 2).reshape(128, DT * DL)
        wv = inputs["Wv"][:, sl].reshape(DT, 128, DL).transpose(1, 0, 2).reshape(128, DT * DL)
        wo = inputs["Wo"][sl, :].reshape(2, 128, D).transpose(1, 0, 2).reshape(128, 2 * D)
        wes = np.concatenate([inputs["We"][:, hsl], inputs["Ws"][:, hsl]], axis=1)  # [1024, 8]
        wes = wes.reshape(DT, 128, 2 * HL).transpose(1, 0, 2).reshape(128, DT * 2 * HL)
        bes = np.stack([inputs["be"][hsl], inputs["bs"][hsl]], axis=1)  # [4, 2]
        maps.append({
            "x": np.ascontiguousarray(x[b].astype(BF16)),
            "wq": np.ascontiguousarray(wq.astype(BF16)),
            "wk": np.ascontiguousarray(wk.astype(BF16)),
            "wv": np.ascontiguousarray(wv.astype(BF16)),
            "wo": np.ascontiguousarray(wo.astype(BF16)),
            "bq2": np.ascontiguousarray(inputs["bq"][sl].reshape(2, 128).T),
            "bk2": np.ascontiguousarray(inputs["bk"][sl].reshape(2, 128).T),
            "bvrow": np.ascontiguousarray(inputs["bv"][sl].reshape(1, DL).astype(BF16)),
            "wes": np.ascontiguousarray(wes),
            "bes": np.ascontiguousarray(bes),
            "gexp": gexp,
            "ident": ident.astype(BF16),
        })
    return maps


def kernel(**inputs):
    if "nc" not in _CACHE:
        _CACHE["nc"] = _build_nc()
    nc = _CACHE["nc"]
    maps = _in_maps({k: np.asarray(v) for k, v in inputs.items()})

    from concourse.bass_utils import run_bass_kernel_spmd

    trace = bool(int(os.environ.get("KERNEL_TRACE", "0")))
    res = run_bass_kernel_spmd(
        nc, maps, list(range(NCORES)), trace=trace,
        tmpdir=os.environ.get("KERNEL_TRACE_DIR") if trace else None,
    )
    _CACHE["last_result"] = res
    bo = np.asarray(inputs["bo"], np.float32)
    out = np.zeros((B, S, D), np.float32)
    for b in range(B):
        acc = np.zeros((S, D), np.float32)
        for g in range(HG):
            acc += res.results[b * HG + g]["out"]
        out[b] = acc + bo
    return out



# revision 37
# speedup vs baseline: 1.2266x; 1.2266x over previous
"""Trainium2 Bass kernel for nn_ASISNativeAttention (B=2,S=2048,D=1024,H=16).

Sharding: 8 cores = 2 batches x 4 head-groups (4 heads each); host splits
inputs per core and sums the 4 partial output projections per batch (+bo).

v2 design (vs the all-bf16 v1): the two elementwise engines (ACT, DVE) are
the roofline -- 131K partition-lines of exp per core -- so every matmul that
can cheaply move to fp8 DoubleRow mode does, freeing PE far below the
elementwise roof, and the exp work is split across BOTH elementwise engines:

  PE    x (fp8, host-transposed) -> q,k,v projections in fp8 DoubleRow
        (2 d-tiles per matmul); scores per head via DoubleRow with a
        stride-0 replicated k-tile pair (computes 2*q.k, folded into the
        exp scale 1/32... -> 1/16 of the x2 psum); PV and out-proj in bf16;
        mean-pool via fp8 DoubleRow ones-column matmuls.
  ACT   exact exp (scale=1/16) psum->bf16 for ~half the score tiles; the
        qk/v/out drains it is assigned; 2 sigmoids.
  DVE   Schraudolph exp for the other tiles: one tensor_scalar
        (psum*A+B -> int16) whose int16 bits ARE the bf16 probs; ctx
        normalize (strided reciprocal + stride-0-broadcast tensor_tensor);
        gated ctxT drain (2x mode); its share of drains.
  Pool  output DMA via SWDGE (keeps HWDGE/SP free); no tensor work (GPSIMD
        cannot access PSUM on TRN2).

Accuracy (numpy study vs reference, same seed): rel err ~1.2e-2 < 2e-2 gate;
fp8 exposure limited to x / Wq,Wk,Wv / stored qT,kT (probs, v, ctx, Wo stay
bf16). Host sums partial outputs in fp32 and adds bo.

kernel.py is self-contained: numpy/ml_dtypes + the concourse tree at
/opt/trn_rl_repo.
"""

import os
import sys
import numpy as np
import ml_dtypes

BF16 = ml_dtypes.bfloat16
F8 = ml_dtypes.float8_e4m3

sys.path.insert(0, "/opt/trn_rl_repo")

B, S, D, H = 2, 2048, 1024, 16
HD = 64          # head dim
NCORES = 8
HG = 4           # head groups = cores per batch
HL = H // HG     # heads per core (4)
DL = D // HG     # local width (256)
ST = S // 128    # 16 s-tiles
DT = D // 128    # 8 d-tiles
SC = 512         # sq chunk width for scores
NSC = S // SC    # 4 chunks

# Schraudolph exp constants: int16 = rint(psum * A16 + B16); psum holds
# 2*q.k so the effective exp argument is psum/16 = q.k/8.
_C_SCH = 360000.0
A16 = float(2.0**7 / np.log(2.0) / 16.0)
B16 = float(127 * 2**7 - _C_SCH / 65536.0)

# --- engine-assignment knobs (tuned against TimelineSim) ---
# exp engine per chunk (16 chars: a-stretch skp 0-7 then b-stretch skp 0-7):
# 'A' = ACT exact exp, 'D' = DVE Schraudolph
EXP_PAT = [
    "AADAADAD" "AADAADAD",   # 10A chunks
    "AADAADAD" "ADAADADD",   # 9A chunks
] * 4
if os.environ.get("KB_ALLACT"):
    EXP_PAT = ["A" * 16] * 8
KB_NOSWDGE = bool(os.environ.get("KB_NOSWDGE"))
KB_CHUNKS = int(os.environ.get("KB_CHUNKS", "8"))
KB_LEADIN = int(os.environ.get("KB_LEADIN", "99"))
# qk projection drain engines, one char per unit (k-i0 x4, q-i0 x4, then 8 i1)
QK_DRAIN = "ADADADADADADADAD"
# v drain engines, one per psum group (8)
V_DRAIN = "ADADADAD"
# out-proj drain engines, one per po half (32)
OUT_DRAIN = "AD" * 16
FILLER_NS = 1200   # filler budget popped per b-stretch skp

_CACHE = {}


def _build_nc():
    import concourse.bass as bass
    import concourse.mybir as mybir
    from concourse.tile import TileContext

    fp32 = mybir.dt.float32
    bf16 = mybir.dt.bfloat16
    f8 = mybir.dt.float8e4
    i16 = mybir.dt.int16
    AF = mybir.ActivationFunctionType
    ALU = mybir.AluOpType
    DRm = mybir.MatmulPerfMode.DoubleRow

    nc = bass.Bass()

    xt8_d = nc.declare_dram_parameter("xt8", [64, 2 * DT * S], f8, isOutput=False)
    x8_d = nc.declare_dram_parameter("x8", [128, ST * D], f8, isOutput=False)
    wq_d = nc.declare_dram_parameter("wq8", [64, 2 * DT * DL], f8, isOutput=False)
    wk_d = nc.declare_dram_parameter("wk8", [64, 2 * DT * DL], f8, isOutput=False)
    wv_d = nc.declare_dram_parameter("wv8", [64, 2 * DT * DL], f8, isOutput=False)
    wo_d = nc.declare_dram_parameter("wo", [128, 2 * D], bf16, isOutput=False)
    bqk_d = nc.declare_dram_parameter("bqk", [128, 4], fp32, isOutput=False)
    bv8_d = nc.declare_dram_parameter("bv8p", [1, 2 * DL], f8, isOutput=False)
    wes_d = nc.declare_dram_parameter("wes", [128, DT * 2 * HL], fp32, isOutput=False)
    bes_d = nc.declare_dram_parameter("bes", [HL, 2], fp32, isOutput=False)
    gexp_d = nc.declare_dram_parameter("gexp", [HL, DL], fp32, isOutput=False)
    id_d = nc.declare_dram_parameter("ident", [128, 128], bf16, isOutput=False)
    out_d = nc.declare_dram_parameter("out", [S, D], bf16, isOutput=True)

    with TileContext(nc) as tc:
        with tc.tile_pool(name="persist", bufs=1) as P:
            xt8 = P.tile([64, 2 * DT * S], f8, tag="xt8")
            wq8 = P.tile([64, 2 * DT * DL], f8, tag="wq8")
            wk8 = P.tile([64, 2 * DT * DL], f8, tag="wk8")
            wv8 = P.tile([64, 2 * DT * DL], f8, tag="wv8")
            wo = P.tile([128, 2 * D], bf16, tag="wo")
            qT8 = P.tile([128, 2 * S], f8, tag="qT8")
            kT8 = P.tile([128, 2 * S], f8, tag="kT8")
            v = P.tile([128, ST * HL * 65], bf16, tag="v")
            ctx = P.tile([128, ST * DL], bf16, tag="ctx")
            ctxT = P.tile([128, 2 * S], bf16, tag="ctxT")
            bqk = P.tile([128, 4], fp32, tag="bqk")
            bv8p = P.tile([1, 2 * DL], f8, tag="bv8p")
            ones82 = P.tile([128, 2, 64], f8, tag="ones82")
            ones8r = P.tile([1, 2 * 128], f8, tag="ones8r")
            z8row = P.tile([1, 2 * 260], f8, tag="z8row")
            wes = P.tile([128, DT * 2 * HL], fp32, tag="wes")
            bes = P.tile([HL, 2], fp32, tag="bes")
            gexp = P.tile([HL, DL], fp32, tag="gexp")
            ident = P.tile([128, 128], bf16, tag="ident")
            ones1 = P.tile([1, 1], fp32, tag="ones1")
            xmrow = P.tile([1, D], fp32, tag="xmrow")
            xm_col = P.tile([128, DT], fp32, tag="xm_col")
            gcol = P.tile([128, 2], fp32, tag="gcol")

            dma = nc.sync.dma_start

            def vview(t):
                return v[:].rearrange("p (t h c) -> p t h c", h=HL, c=65)[:, t]

            nc.vector.memset(ones82[:], 1.0)
            nc.vector.memset(ones8r[:], 1.0)
            nc.vector.memset(z8row[:], 0.0)
            nc.vector.memset(ones1[:], 1.0)
            # constant softmax-denominator columns of v
            nc.vector.memset(
                v[:].rearrange("p (t h c) -> p t h c", h=HL, c=65)[:, :, :, 64:65], 1.0
            )

            xt8v = xt8[:].rearrange("p (j s) -> p j s", s=S)      # [64, 16, S]
            wq8v = wq8[:].rearrange("p (j m) -> p j m", m=DL)     # [64, 16, DL]
            wk8v = wk8[:].rearrange("p (j m) -> p j m", m=DL)
            wv8v = wv8[:].rearrange("p (j m) -> p j m", m=DL)

            with (
                tc.tile_pool(name="x8l", bufs=8) as XL,
                tc.tile_pool(name="pm", bufs=3, space="PSUM") as PM,
                tc.tile_pool(name="pcps", bufs=2, space="PSUM") as PCS,
                tc.tile_pool(name="ets", bufs=26) as EX,
                tc.tile_pool(name="rc", bufs=8) as RC,
                tc.tile_pool(name="ob", bufs=2) as OB,
                tc.tile_pool(name="gs", bufs=1) as GS,
            ):
                xmp = [None, None]

                xbs = []

                def load_x_pair(pr):
                    """DMA natural-x pair pr via SWDGE (Pool) off the HWDGE path."""
                    xb = XL.tile([128, 2, D], f8, tag="xb", name=f"xb{pr}")
                    eng = nc.scalar if KB_NOSWDGE else nc.gpsimd
                    eng.dma_start(
                        out=xb[:],
                        in_=x8_d[:].rearrange("p (r c d) -> p r c d", c=2, d=D)[:, pr])
                    xbs.append(xb)

                def meanpool_mm(pr, xmps):
                    lhs = ones82[:]  # [128, 2, 64]: dual-fp8 ldweights needs wide M
                    for qh in range(4):
                        half, qq = qh // 2, qh % 2
                        first = (pr == 0 and qq == 0)
                        nc.tensor.matmul(
                            xmps[half][:, qq * 256:(qq + 1) * 256],
                            lhsT=lhs,
                            rhs=xbs[pr][:, :, qh * 256:(qh + 1) * 256],
                            start=first, stop=first,
                            skip_group_check=not first,
                            perf_mode=DRm,
                        )

                def qk_unit(w8v, dst8, bcol, i, sc, eng):
                    """Project one [128, SC] chunk of qT or kT (fp8 out)."""
                    pp = PM.tile([128, SC], fp32, tag="pm",
                                 name=f"pp{dst8.tensor.name}_{i}_{sc}")
                    for qq in range(2):
                        # x / weights live on 64 partitions x 16 d-tiles:
                        # dual-fp8 DR caps 2*K*M at the PE array size and the
                        # PE hangs if ldweights base-partition changes inside
                        # an accumulation chain, so every chain stays K=64.
                        for dp in range(DT):
                            first = (qq == 0 and dp == 0)
                            nc.tensor.matmul(
                                pp[:, qq * 256:(qq + 1) * 256],
                                lhsT=w8v[:, 2 * dp:2 * dp + 2, i * 128:(i + 1) * 128],
                                rhs=xt8v[:, 2 * dp:2 * dp + 2,
                                         sc * SC + qq * 256: sc * SC + (qq + 1) * 256],
                                start=first, stop=first,
                                skip_group_check=not first,
                                perf_mode=DRm,
                            )
                    dst = dst8[:, i * S + sc * SC: i * S + (sc + 1) * SC]
                    if eng == "A":
                        nc.scalar.activation(dst, pp[:], AF.Identity, bias=bcol)
                    else:
                        nc.vector.tensor_scalar(
                            out=dst, in0=pp[:], scalar1=bcol, scalar2=None,
                            op0=ALU.add,
                        )

                def v_unit(g, eng):
                    """Project v for s-tiles 2g, 2g+1 (one psum bank)."""
                    pv = PM.tile([128, 2 * DL], fp32, tag="pm", name=f"pv{g}")
                    for t2 in range(2):
                        t = 2 * g + t2
                        sl = pv[:, t2 * DL:(t2 + 1) * DL]
                        nc.tensor.matmul(
                            sl, lhsT=ones8r[:].rearrange("o (c m) -> o c m", c=2),
                            rhs=bv8p[:].rearrange("o (c m) -> o c m", c=2),
                            start=(t2 == 0), stop=(t2 == 0),
                            skip_group_check=(t2 == 1), perf_mode=DRm,
                        )
                        for dp in range(DT):
                            nc.tensor.matmul(
                                sl,
                                lhsT=xt8v[:, 2 * dp:2 * dp + 2, t * 128:(t + 1) * 128],
                                rhs=wv8v[:, 2 * dp:2 * dp + 2, :],
                                start=False, stop=False,
                                skip_group_check=True,
                                perf_mode=DRm,
                            )
                    for t2 in range(2):
                        t = 2 * g + t2
                        src = pv[:, t2 * DL:(t2 + 1) * DL].rearrange(
                            "p (h c) -> p h c", c=HD)
                        dst = vview(t)[:, :, 0:HD]
                        if eng == "A":
                            nc.scalar.copy(dst, src)
                        else:
                            nc.vector.tensor_copy(dst, src)

                def score_exp(i, sc, hh, skp, eng):
                    """Scores for sk-tiles (2skp, 2skp+1) x [sc*SC, (sc+1)*SC) of
                    head 2i+hh; one wide exp. psum holds 2*q.k (stride-0 DR)."""
                    r = hh * 64
                    sp = PM.tile([128, 2 * SC], fp32, tag="pm",
                                 name=f"sp{i}_{sc}_{hh}_{skp}")
                    for half in range(2):
                        sk = 2 * skp + half
                        lhsT = kT8[r:r + 64, i * S + sk * 128: i * S + (sk + 1) * 128] \
                            .unsqueeze(1).broadcast_to([64, 2, 128])
                        for qq in range(2):
                            rhs = qT8[r:r + 64,
                                      i * S + sc * SC + qq * 256: i * S + sc * SC + (qq + 1) * 256] \
                                .unsqueeze(1).broadcast_to([64, 2, 256])
                            # qq0 opens the bank's zero region; qq1 assigns
                            # into still-pending bytes (no second group)
                            nc.tensor.matmul(
                                sp[:, half * SC + qq * 256: half * SC + (qq + 1) * 256],
                                lhsT=lhsT, rhs=rhs, start=(qq == 0), stop=(qq == 0),
                                skip_group_check=(qq == 1),
                                perf_mode=DRm,
                            )
                    if eng == "A":
                        et = EX.tile([128, 2 * SC], bf16, tag="et",
                                     name=f"et{i}_{sc}_{hh}_{skp}")
                        nc.scalar.activation(et[:], sp[:], AF.Exp, scale=1.0 / 16.0)
                        return et[:]
                    et = EX.tile([128, 2 * SC], i16, tag="et",
                                 name=f"et{i}_{sc}_{hh}_{skp}")
                    nc.vector.tensor_scalar(
                        out=et[:], in0=sp[:], scalar1=A16, scalar2=B16,
                        op0=ALU.mult, op1=ALU.add,
                    )
                    return et[:].bitcast(bf16)

                def pv_mm(cps, ets, i, hh, sk):
                    h = 2 * i + hh
                    skp, half = sk // 2, sk % 2
                    for u in range(SC // 128):
                        nc.tensor.matmul(
                            cps[u][:, hh * 65:(hh + 1) * 65],
                            lhsT=ets[skp][:, half * SC + u * 128: half * SC + (u + 1) * 128],
                            rhs=v[:, sk * HL * 65 + h * 65: sk * HL * 65 + (h + 1) * 65],
                            start=False, stop=False, skip_group_check=True,
                        )

                def outproj_units(t):
                    def unit():
                        ot = OB.tile([128, D], bf16, tag="ot", name=f"ot{t}")
                        po = PM.tile([128, D], fp32, tag="pm", name=f"po{t}")
                        for n2 in range(2):
                            for i2 in range(2):
                                nc.tensor.matmul(
                                    po[:, n2 * 512:(n2 + 1) * 512],
                                    lhsT=ctxT[:, i2 * S + t * 128: i2 * S + (t + 1) * 128],
                                    rhs=wo[:, i2 * D + n2 * 512: i2 * D + (n2 + 1) * 512],
                                    start=(i2 == 0), stop=(i2 == 1),
                                )
                        if OUT_DRAIN[t % len(OUT_DRAIN)] == "A":
                            nc.scalar.copy(ot[:], po[:])
                        else:
                            nc.vector.tensor_copy(ot[:], po[:])
                        (nc.scalar if KB_NOSWDGE else nc.gpsimd).dma_start(
                            out=out_d[t * 128:(t + 1) * 128, :], in_=ot[:])
                    return [(1000, unit)]

                fillers = []

                def pop_fillers(budget_ns):
                    spent = 0
                    while fillers and spent < budget_ns:
                        ns, unit = fillers.pop(0)
                        unit()
                        spent += ns

                # ---------------- lead-in ----------------
                # xt8 chunk 0 + wk8 first: they gate the whole pipeline
                def dma_xt(sc):
                    dma(xt8v[:, :, sc * SC:(sc + 1) * SC],
                        xt8_d[:].rearrange("p (j s) -> p j s", s=S)[:, :, sc * SC:(sc + 1) * SC])
                dma_xt(0)
                dma(wk8[:], wk_d[:])
                dma(bqk[:], bqk_d[:])
                dma(wq8[:], wq_d[:])
                dma_xt(1)
                dma(ident[:], id_d[:])
                dma_xt(2)
                dma_xt(3)
                dma(wv8[:], wv_d[:])
                dma(bv8p[:], bv8_d[:])
                dma(wes[:], wes_d[:])
                dma(bes[:], bes_d[:])
                dma(gexp[:], gexp_d[:])
                dma(wo[:], wo_d[:])
                for pr in range(ST // 2):
                    load_x_pair(pr)

                # minimal critical path: k i=0 sc=0 + q i=0 sc=0 lets chunk-0
                # scores start; the other k i=0 chunks interleave with the
                # first a-stretch (k-sc j emitted just before skp 2j).
                if KB_LEADIN >= 1:
                    qk_unit(wk8v, kT8, bqk[:, 2:3], 0, 0, QK_DRAIN[0])
                if KB_LEADIN >= 2:
                    qk_unit(wq8v, qT8, bqk[:, 0:1], 0, 0, QK_DRAIN[4])

                # deferred to fillers: all of i=1 (needed from chunk 4)
                for sc in range(NSC):
                    fillers.append((1100, (lambda s: lambda: qk_unit(
                        wk8v, kT8, bqk[:, 3:4], 1, s, QK_DRAIN[(8 + s) % 16]))(sc)))
                for sc in range(NSC):
                    fillers.append((1100, (lambda s: lambda: qk_unit(
                        wq8v, qT8, bqk[:, 1:2], 1, s, QK_DRAIN[(12 + s) % 16]))(sc)))

                def gates_block():
                    xcp = PCS.tile([128, DT], fp32, tag="cp", name="xcp")
                    for hf in range(2):
                        nc.vector.tensor_copy(
                            xmrow[:, hf * 512:(hf + 1) * 512], xmp[hf][0:1, :])
                    for j in range(DT):
                        nc.tensor.matmul(
                            xcp[:, j: j + 1],
                            lhsT=xmrow[:, j * 128:(j + 1) * 128],
                            rhs=ones1[:],
                            start=True, stop=True,
                        )
                    nc.vector.tensor_copy(xm_col[:], xcp[:])
                    gpe = PCS.tile([HL, 1], fp32, tag="cp", name="gpe")
                    gps = PCS.tile([HL, 1], fp32, tag="cp", name="gps")
                    for j in range(DT):
                        nc.tensor.matmul(
                            gpe[:], lhsT=wes[:, j * 8: j * 8 + 4],
                            rhs=xm_col[:, j: j + 1],
                            start=(j == 0), stop=(j == DT - 1),
                        )
                    for j in range(DT):
                        nc.tensor.matmul(
                            gps[:], lhsT=wes[:, j * 8 + 4: j * 8 + 8],
                            rhs=xm_col[:, j: j + 1],
                            start=(j == 0), stop=(j == DT - 1),
                        )
                    eth = GS.tile([HL, 1], fp32, tag="eth")
                    saf = GS.tile([HL, 1], fp32, tag="saf")
                    gate = GS.tile([HL, 1], fp32, tag="gate")
                    nc.scalar.activation(eth[:], gpe[:], AF.Sigmoid,
                                         bias=bes[:, 0:1], scale=1.0 / S)
                    nc.scalar.activation(saf[:], gps[:], AF.Sigmoid,
                                         bias=bes[:, 1:2], scale=1.0 / S)
                    nc.vector.tensor_mul(gate[:], eth[:], saf[:])
                    for i in range(2):
                        pgc = PCS.tile([128, 1], fp32, tag="cp", name=f"pgc{i}")
                        nc.tensor.matmul(
                            pgc[:], lhsT=gexp[:, i * 128:(i + 1) * 128], rhs=gate[:],
                            start=True, stop=True,
                        )
                        nc.vector.tensor_copy(gcol[:, i: i + 1], pgc[:])

                # ---------------- chunk loop ----------------
                def alloc_cps(i, sc):
                    # two 1-bank tiles, each holding two u-slots of [128, 130];
                    # a zero rank-1 matmul opens each bank's zero region so the
                    # interleaved PV accumulations need no group bookkeeping
                    pair = [PCS.tile([128, 2, 130], fp32, tag="cp",
                                     name=f"cp{i}_{sc}_{w}") for w in range(2)]
                    for w in range(2):
                        for a2 in range(2):
                            nc.tensor.matmul(
                                pair[w][:, a2, :],
                                lhsT=ones8r[:].rearrange("o (c m) -> o c m", c=2),
                                rhs=z8row[:].rearrange("o (c m) -> o c m", c=2)[:, :, 0:130],
                                start=True, stop=True,
                                skip_group_check=(a2 == 1),
                                perf_mode=DRm,
                            )
                    return [pair[u // 2][:, u % 2, :] for u in range(SC // 128)]

                def pv_mm_u(cps, ets, i, hh, u):
                    h = 2 * i + hh
                    for sk in range(ST):
                        skp, half = sk // 2, sk % 2
                        nc.tensor.matmul(
                            cps[u][:, hh * 65:(hh + 1) * 65],
                            lhsT=ets[skp][:, half * SC + u * 128: half * SC + (u + 1) * 128],
                            rhs=v[:, sk * HL * 65 + h * 65: sk * HL * 65 + (h + 1) * 65],
                            start=(sk == 0), stop=(sk == ST - 1),
                        )

                def make_tail_parts(i, sc, cps, last):
                    """Staggered per-u closures: partN(u) = DVE normalize only;
                    partT(u) = PE transpose (+ drain at odd u), emitted one
                    slot later so the transpose never queues on PE before its
                    normalize has finished on DVE. PV ran in the b-stretch."""
                    state = {"tp": None}

                    def partN(u):
                        def f():
                            t = sc * (SC // 128) + u
                            rec2 = RC.tile([128, 2], fp32, tag="rec",
                                           name=f"rec{i}_{t}")
                            cpv = cps[u].rearrange("p (h c) -> p h c", c=65)
                            nc.vector.reciprocal(rec2[:], cpv[:, :, 64])
                            with nc.allow_low_precision("softmax-normalized bf16 ctx"):
                                nc.vector.tensor_tensor(
                                    out=ctx[:, t * DL + i * 128: t * DL + (i + 1) * 128]
                                        .rearrange("p (h c) -> p h c", c=HD),
                                    in0=cpv[:, :, 0:HD],
                                    in1=rec2[:].unsqueeze(2).broadcast_to([128, 2, HD]),
                                    op=ALU.mult,
                                )
                        return f

                    def partT(u):
                        def f():
                            if u % 2 == 0:
                                state["tp"] = PM.tile([128, 256], bf16, tag="pm",
                                                      name=f"tp{i}_{sc}_{u // 2}")
                            tp = state["tp"]
                            t = sc * (SC // 128) + u
                            nc.tensor.transpose(
                                tp[:, (u % 2) * 128:(u % 2 + 1) * 128],
                                ctx[:, t * DL + i * 128: t * DL + (i + 1) * 128],
                                ident[:],
                            )
                            if last:
                                # per-u drain so the final out-projs pipeline
                                nc.vector.tensor_scalar(
                                    out=ctxT[:, i * S + t * 128: i * S + (t + 1) * 128],
                                    in0=tp[:, (u % 2) * 128:(u % 2 + 1) * 128],
                                    scalar1=gcol[:, i: i + 1],
                                    scalar2=None,
                                    op0=ALU.mult,
                                )
                                for _, unit in outproj_units(t):
                                    unit()
                            elif u % 2 == 1:
                                nc.vector.tensor_scalar(
                                    out=ctxT[:, i * S + (t - 1) * 128: i * S + (t + 1) * 128],
                                    in0=tp[:],
                                    scalar1=gcol[:, i: i + 1],
                                    scalar2=None,
                                    op0=ALU.mult,
                                )
                                if u == SC // 128 - 1 and i == 1:
                                    for t2 in range(sc * 4, sc * 4 + 4):
                                        fillers.extend(outproj_units(t2))
                        return f

                    parts = []
                    nn = [partN(u) for u in range(SC // 128)]
                    tt = [partT(u) for u in range(SC // 128)]
                    parts.append(nn[0])
                    for u in range(1, SC // 128):
                        parts.append(lambda a=nn[u], b=tt[u - 1]: (a(), b()))
                    parts.append(tt[SC // 128 - 1])
                    return parts

                nchunks = [(i, sc) for i in range(2) for sc in range(NSC)][:KB_CHUNKS]
                pending_parts = []
                for n, (i, sc) in enumerate(nchunks):
                    first = (n == 0)
                    if first:
                        # mean-pool runs here: PE is otherwise idle during the
                        # first a-stretch and the PC psum pool is free.
                        xmp[0] = PCS.tile([64, 512], fp32, tag="cp", name="xmp0")
                        xmp[1] = PCS.tile([64, 512], fp32, tag="cp", name="xmp1")
                    ets_a = []
                    for skp in range(ST // 2):
                        if first and skp in (2, 4, 6):
                            # k i=0 chunk j just ahead of the scores needing it
                            qk_unit(wk8v, kT8, bqk[:, 2:3], 0, skp // 2,
                                    QK_DRAIN[skp // 2])
                        ets_a.append(score_exp(i, sc, 0, skp,
                                               EXP_PAT[n % len(EXP_PAT)][skp]))
                        if skp >= 1 and pending_parts:
                            pending_parts.pop(0)()
                        elif skp >= 2 and not first:
                            pop_fillers(FILLER_NS)
                    while pending_parts:
                        pending_parts.pop(0)()
                    if first:
                        # keep these off the critical lead-in: the scheduler
                        # would otherwise hoist them ahead of the k/q units
                        # and head-block PE on the slow x8-pair DMAs.
                        with tc.tile_wait_until(0.012):
                            for pr in range(ST // 2):
                                meanpool_mm(pr, xmp)
                        with tc.tile_wait_until(0.014):
                            gates_block()
                    cps = alloc_cps(i, sc)
                    ets_b = []
                    for skp in range(ST // 2):
                        ets_b.append(score_exp(i, sc, 1, skp,
                                               EXP_PAT[n % len(EXP_PAT)][8 + skp]))
                        if first:
                            v_unit(skp, V_DRAIN[skp % 8])
                        # PV for both heads streams through the b-stretch.
                        # Head b lags one skp so PE never queues behind the
                        # exp that was just issued for this skp.
                        pv_mm(cps, ets_a, i, 0, 2 * skp)
                        pv_mm(cps, ets_a, i, 0, 2 * skp + 1)
                        if skp >= 1:
                            pv_mm(cps, ets_b, i, 1, 2 * (skp - 1))
                            pv_mm(cps, ets_b, i, 1, 2 * (skp - 1) + 1)
                        if skp == 5 and i == 0 and sc < NSC - 1:
                            # q i=0 chunk sc+1 mid-b-stretch, off the boundary
                            qk_unit(wq8v, qT8, bqk[:, 0:1], 0, sc + 1,
                                    QK_DRAIN[4 + sc + 1])
                        if not first:
                            pop_fillers(FILLER_NS)
                    pending_parts = make_tail_parts(
                        i, sc, cps, last=(n == len(nchunks) - 1))
                    # the last head-b PV pair rides into the next a-stretch so
                    # the chunk boundary never waits on the final exp
                    def last_pv(cps=cps, ets_b=ets_b, i=i):
                        pv_mm(cps, ets_b, i, 1, ST - 2)
                        pv_mm(cps, ets_b, i, 1, ST - 1)
                    pending_parts.insert(0, last_pv)
                if KB_CHUNKS == 8:
                    for p in pending_parts:
                        p()
                    pop_fillers(10**9)
                else:
                    pending_parts.clear()
                    fillers.clear()
                    # touch out so the output DMA graph exists
                    ot = OB.tile([128, D], bf16, tag="ot", name="ot_stub")
                    nc.vector.memset(ot[:], 0.0)
                    nc.sync.dma_start(out=out_d[0:128, :], in_=ot[:])

    _split_multi_waits(nc)
    return nc


def _split_multi_waits(nc, skip=("InstEventSemaphore",)):
    """Hoist extra sync waits onto preceding same-engine NoOps.

    Walrus codegen can attach only one sync wait to some instruction
    encodings, so any instruction carrying N>1 waits is rewritten as N-1
    single-wait NoOps followed by the instruction with the last wait.
    """
    import concourse.mybir as mybir

    eng = {
        "EngineType.PE": nc.tensor,
        "EngineType.DVE": nc.vector,
        "EngineType.Activation": nc.scalar,
        "EngineType.Pool": nc.gpsimd,
        "EngineType.SP": nc.sync,
    }

    def fresh_nop(engine_key):
        nop = eng[engine_key].nop(hint="wsplit").ins
        for fn in nc.m.functions:
            for bb in fn.blocks:
                for i, ins in enumerate(bb.instructions):
                    if ins.name == nop.name:
                        del bb.instructions[i]
                        return nop
        raise RuntimeError("fresh nop not found")

    for fn in nc.m.functions:
        for bb in fn.blocks:
            insertions = []
            for idx, ins in enumerate(bb.instructions):
                if type(ins).__name__ in skip:
                    continue
                si = ins.sync_info
                if si is None or len(si.on_wait) <= 1:
                    continue
                waits = list(si.on_wait)
                nops = []
                for w in waits[:-1]:
                    nop = fresh_nop(str(ins.engine))
                    nop.sync_info = mybir.SyncInfo(on_wait=[w], on_update=[])
                    nops.append(nop)
                ins.sync_info = mybir.SyncInfo(
                    on_wait=[waits[-1]], on_update=list(si.on_update)
                )
                insertions.append((idx, nops))
            for idx, nops in reversed(insertions):
                bb.instructions[idx:idx] = nops


def _in_maps(inputs):
    x = np.ascontiguousarray(inputs["x"], np.float32)
    maps = []
    ident = np.eye(128, dtype=np.float32)
    gexp = np.zeros((HL, DL), np.float32)
    for h in range(HL):
        gexp[h, h * HD:(h + 1) * HD] = 1.0
    x8 = x.astype(F8)          # [B, S, D]
    for c in range(NCORES):
        b, g = c // HG, c % HG
        sl = slice(g * DL, (g + 1) * DL)
        hsl = slice(g * HL, (g + 1) * HL)
        # xT fp8 on 64 partitions: [64, jj, s] = x[b][s, jj*64+p]
        xt8 = np.ascontiguousarray(
            x8[b].T.reshape(2 * DT, 64, S).transpose(1, 0, 2).reshape(64, 2 * DT * S))
        # natural-x pairs: [128, pr, c2, col] = x[b][pr*256 + c2*128 + p, col]
        x8n = np.ascontiguousarray(
            x8[b].reshape(ST // 2, 2, 128, D).transpose(2, 0, 1, 3).reshape(128, ST * D))
        def wtile(w):
            return np.ascontiguousarray(
                w[:, sl].reshape(2 * DT, 64, DL).transpose(1, 0, 2).reshape(64, 2 * DT * DL).astype(F8))
        wo = inputs["Wo"][sl, :].reshape(2, 128, D).transpose(1, 0, 2).reshape(128, 2 * D)
        bqk = np.stack([
            inputs["bq"][sl][0:128], inputs["bq"][sl][128:256],
            inputs["bk"][sl][0:128], inputs["bk"][sl][128:256],
        ], axis=1).astype(np.float32)
        bv8p = np.zeros((1, 2 * DL), F8)
        bv8p[0, 0:DL] = inputs["bv"][sl].astype(F8)
        wes = np.concatenate([inputs["We"][:, hsl], inputs["Ws"][:, hsl]], axis=1)
        wes = wes.reshape(DT, 128, 2 * HL).transpose(1, 0, 2).reshape(128, DT * 2 * HL)
        bes = np.stack([inputs["be"][hsl], inputs["bs"][hsl]], axis=1)
        maps.append({
            "xt8": xt8,
            "x8": x8n,
            "wq8": wtile(inputs["Wq"]),
            "wk8": wtile(inputs["Wk"]),
            "wv8": wtile(inputs["Wv"]),
            "wo": np.ascontiguousarray(wo.astype(BF16)),
            "bqk": np.ascontiguousarray(bqk),
            "bv8p": bv8p,
            "wes": np.ascontiguousarray(wes.astype(np.float32)),
            "bes": np.ascontiguousarray(bes.astype(np.float32)),
            "gexp": gexp,
            "ident": ident.astype(BF16),
        })
    return maps


def kernel(**inputs):
    if "nc" not in _CACHE:
        _CACHE["nc"] = _build_nc()
    nc = _CACHE["nc"]
    maps = _in_maps({k: np.asarray(v) for k, v in inputs.items()})

    from concourse.bass_utils import run_bass_kernel_spmd

    trace = bool(int(os.environ.get("KERNEL_TRACE", "0")))
    res = run_bass_kernel_spmd(
        nc, maps, list(range(NCORES)), trace=trace,
        tmpdir=os.environ.get("KERNEL_TRACE_DIR") if trace else None,
    )
    _CACHE["last_result"] = res
    bo = np.asarray(inputs["bo"], np.float32)
    out = np.zeros((B, S, D), np.float32)
    for b in range(B):
        acc = np.zeros((S, D), np.float32)
        for g in range(HG):
            acc += res.results[b * HG + g]["out"].astype(np.float32)
        out[b] = acc + bo
    return out
